# revision 18
# baseline (speedup 1.0000x reference)
"""ChebyKAN Trainium2 kernel.

Reference computation:
    t = tanh(x)                      # x: [8192, 768]
    cheby[b,i,d] = T_d(t[b,i])       # Chebyshev polys, d = 0..8
    out[b,j] = sum_{i,d} cheby[b,i,d] * coefficients[i,j,d]

Strategy (data-parallel over batch across 8 cores):
  - Each core gets a 1024-row batch shard, transposed on host to xt [768, 1024]
    so the contraction dim (in_features) lands on SBUF partitions.
  - out.T[j, b] = sum_k coeffK[k, j] * chebyK[k, b], K = 6*128 i-tiles x 8
    degrees (d=0 contributes a j-constant folded into a bias at PSUM drain).
  - bf16 matmuls: HW-measured 216 ns per K=128,N=512 matmul vs 227 ns for
    f32r (5% faster slot), quantization error ~2e-3 l2 vs the 2e-2 gate.
    fp8 DoubleRow (2x) was measured too but single-pass e4m3 error is
    3.3e-2 -> fails the gate; residual schemes cost 2x slots = no win.
  - Coefficients are cast to bf16 on host and kept SBUF-RESIDENT: one
    9.4 MB stream at kernel start (72 KB/partition) instead of streaming
    f32r twice (38 MB); DMA pressure is then trivial.
  - Two passes over batch halves of 512: per pass, all 6 j-tiles accumulate
    in 6 single-bank PSUM tiles over the 48 K-tiles; 576 matmuls total.
  - Chebyshev tiles via product identities: T2=2t^2-1, T3=2tT2-t, T4=2T2^2-1,
    T5=2T2T3-t, T6=2T3^2-1, T7=2T3T4-t, T8=2T4^2-1.  The even chain
    (t -> sq -> T2 -> sq2 -> T4 -> sq4 -> T8b) runs entirely on the Scalar
    engine with the x2-1 affines folded into activation scale/bias
    (func(in*2 - 1)), so the latency-critical recurrence has zero
    cross-engine hops; odd degrees + T6 run on Vector with bf16 outputs
    written directly by the final fused op.  The f32 recurrence keeps
    full precision; only the matmul operands are bf16.  No GpSimd
    compute (concurrent GpSimd elementwise steals DVE ports).
  - PE is warmed with a few dummy matmuls so HAM/p-state ramp happens
    before real work; real matmuls start as soon as tanh of the first
    tile lands (~9 us, incl. the ~7 us fixed NEFF prologue).
  - Scheduler sensitivity (HW-measured): bumping any tile-pool bufs
    beyond (work=3, cheb=2) reorders the Tile schedule and regresses
    3-6 us (drain/chain priority inversions); the ~430 ns/block slip of
    the cheby producers against the matmul stream is the stable
    equilibrium of the coupled in-order queues and costs ~2.4 us total.
    Device clock note: some runs execute at 2.0 GHz instead of 2.4
    (~172 us instead of ~142); this is environmental (power/thermal),
    not kernel-dependent -- rerun after idle to confirm regressions.
"""

import sys

for _p in ("/opt/trn_rl_repo",):
    if _p not in sys.path:
        sys.path.insert(0, _p)

import ml_dtypes
import numpy as np

import concourse.bass as bass
import concourse.mybir as mybir
import concourse.tile as tile
from concourse import bacc
from concourse import bass_utils
from concourse.tile import TileContext

F32 = mybir.dt.float32
BF16 = mybir.dt.bfloat16
AF = mybir.ActivationFunctionType
OP = mybir.AluOpType

B, I, J, D1 = 8192, 768, 768, 9  # batch, in_features, out_features, degree+1
NCORES = 8
BPC = B // NCORES      # 1024 batch rows per core
IT = I // 128          # 6 i-tiles
KT = IT * 8            # 48 K-tiles (d = 1..8)
JT = J // 128          # 6 j-tiles
HB = 512               # half-batch (matmul N)

_CACHE = {}


def _build_nc():
    nc = bacc.Bacc("TRN2", target_bir_lowering=False, debug=False,
                   num_devices=NCORES)
    xt = nc.dram_tensor("xt", [I, BPC], F32, kind="ExternalInput").ap()
    # coeff[k, i, j]: K-tile k = it*8 + (d-1)
    coeff = nc.dram_tensor("coeff", [KT, 128, J], BF16,
                           kind="ExternalInput").ap()
    bias = nc.dram_tensor("bias", [128, JT], F32, kind="ExternalInput").ap()
    out = nc.dram_tensor("out", [J, BPC], F32, kind="ExternalOutput").ap()

    with TileContext(nc) as tc:
        with (
            tc.tile_pool(name="xtp", bufs=1) as xt_pool,
            tc.tile_pool(name="work", bufs=3) as work,
            tc.tile_pool(name="chebp", bufs=2) as cheb,
            tc.tile_pool(name="coeffp", bufs=1) as coeff_pool,
            tc.tile_pool(name="outp", bufs=6) as out_pool,
            tc.tile_pool(name="biasp", bufs=1) as bias_pool,
            tc.tile_pool(name="psum", bufs=8, space="PSUM") as psum_pool,
        ):
            # PE warm-up scratch (HAM/p-state ramp before real matmuls).
            warm = work.tile([128, HB], BF16, name="warm", tag="warm", bufs=1)
            nc.vector.memset(warm, 0.0)

            bias_all = bias_pool.tile([128, JT], F32, name="bias_all",
                                      tag="bias_all")
            neg1 = bias_pool.tile([128, 1], F32, name="neg1", tag="neg1")
            nc.vector.memset(neg1, -1.0)

            # Resident coefficient tiles + input tiles; DMA order = first
            # consumed first.  The first xt tile is split so the half-0
            # columns (what the first tanh reads) land in ~0.7 us instead
            # of 1.4, and coeff k=0 rides right behind them.
            xt_tiles = [None] * IT
            ct = [None] * KT

            def coeff_tile(k):
                c = coeff_pool.tile([128, J], BF16, name=f"c{k}", tag=f"c{k}")
                nc.sync.dma_start(c, coeff[k])
                ct[k] = c

            xt0 = xt_pool.tile([128, BPC], F32, name="xtt0", tag="xtt0")
            nc.sync.dma_start(xt0[:, :HB], xt[:128, :HB])
            xt_tiles[0] = xt0
            coeff_tile(0)
            nc.sync.dma_start(xt0[:, HB:], xt[:128, HB:])
            for dm1 in range(1, 8):
                coeff_tile(dm1)
            for it in range(1, IT):
                xtt = xt_pool.tile([128, BPC], F32, name=f"xtt{it}",
                                   tag=f"xtt{it}")
                nc.sync.dma_start(xtt, xt[it * 128:(it + 1) * 128, :])
                xt_tiles[it] = xtt
                for dm1 in range(8):
                    coeff_tile(it * 8 + dm1)
                if it == 1:
                    nc.sync.dma_start(bias_all, bias)

            for half in range(2):
                hs = slice(half * HB, (half + 1) * HB)
                ps = [psum_pool.tile([128, HB], F32, name="ps", tag="ps")
                      for _ in range(JT)]
                if half == 0:
                    # dummy matmuls into ps[0]; overwritten by the real
                    # k==0 matmul (start=True clears has_written).  5 full-
                    # width warmups bridge most of the data-delivery window,
                    # then 4 narrow (128-col) ones keep the PE busy with
                    # fine granularity so the handoff to the first real
                    # matmul isn't quantized to 427 ns slots.
                    for _ in range(5):
                        nc.tensor.matmul(ps[0], lhsT=warm[:, :128], rhs=warm,
                                         start=True, stop=True)
                    for _ in range(4):
                        nc.tensor.matmul(ps[0][:, :128], lhsT=warm[:, :128],
                                         rhs=warm[:, :128],
                                         start=True, stop=True)

                for it in range(IT):
                    xs = xt_tiles[it][:, hs]
                    # In steady state the f32 chain (t -> squares) is the
                    # critical path, so emit tanh->t first; tb (consumed at
                    # the very start of the block) has a whole block of
                    # production slack -- except for the first block, where
                    # tb gates the first real matmul.
                    tb = cheb.tile([128, HB], BF16, name="tb", tag="tb")
                    t = work.tile([128, HB], F32, name="t", tag="t")
                    if half == 0 and it == 0:
                        nc.scalar.activation(tb, xs, AF.Tanh)
                        nc.scalar.activation(t, xs, AF.Tanh)
                    else:
                        nc.scalar.activation(t, xs, AF.Tanh)
                        nc.scalar.activation(tb, xs, AF.Tanh)
                    # Even chain runs entirely on the Scalar engine with the
                    # x2-1 affines folded into activation scale/bias --
                    # t -> sq -> T2 -> sq2 -> T4 -> sq4 back-to-back on one
                    # in-order queue, no cross-engine hops.
                    sq = work.tile([128, HB], F32, name="sq", tag="sq")
                    nc.scalar.activation(sq, t, AF.Square)
                    T2 = work.tile([128, HB], F32, name="T2", tag="T2")
                    T2b = cheb.tile([128, HB], BF16, name="T2b", tag="T2b")
                    if half == 0 and it == 0:
                        # Block 0 races its own consumption: the bf16 tap
                        # (consumed at dm1=1, ~1.3 us in) goes first.
                        nc.scalar.activation(T2b, sq, AF.Identity, bias=neg1,
                                             scale=2.0)
                        nc.scalar.activation(T2, sq, AF.Identity, bias=neg1,
                                             scale=2.0)
                    else:
                        nc.scalar.activation(T2, sq, AF.Identity, bias=neg1,
                                             scale=2.0)
                        nc.scalar.activation(T2b, sq, AF.Identity, bias=neg1,
                                             scale=2.0)
                    sq2 = work.tile([128, HB], F32, name="sq2", tag="sq2")
                    nc.scalar.activation(sq2, T2, AF.Square)
                    T4 = work.tile([128, HB], F32, name="T4", tag="T4")
                    nc.scalar.activation(T4, sq2, AF.Identity, bias=neg1,
                                         scale=2.0)
                    T4b = cheb.tile([128, HB], BF16, name="T4b", tag="T4b")
                    nc.scalar.activation(T4b, sq2, AF.Identity, bias=neg1,
                                         scale=2.0)
                    sq4 = work.tile([128, HB], F32, name="sq4", tag="sq4")
                    nc.scalar.activation(sq4, T4, AF.Square)
                    T8b = cheb.tile([128, HB], BF16, name="T8b", tag="T8b")
                    nc.scalar.activation(T8b, sq4, AF.Identity, bias=neg1,
                                         scale=2.0)
                    # Odd degrees + T6 on the Vector engine.
                    # T3 = 2 t T2 - t
                    P = work.tile([128, HB], F32, name="P", tag="P")
                    nc.vector.tensor_mul(P, t, T2)
                    T3 = work.tile([128, HB], F32, name="T3", tag="T3")
                    nc.vector.scalar_tensor_tensor(T3, P, 2.0, t, OP.mult,
                                                   OP.subtract)
                    T3b = cheb.tile([128, HB], BF16, name="T3b", tag="T3b")
                    nc.vector.tensor_copy(T3b, T3)
                    # T5 = 2 T2 T3 - t  (bf16 out)
                    P = work.tile([128, HB], F32, name="P", tag="P")
                    nc.vector.tensor_mul(P, T2, T3)
                    T5b = cheb.tile([128, HB], BF16, name="T5b", tag="T5b")
                    nc.vector.scalar_tensor_tensor(T5b, P, 2.0, t, OP.mult,
                                                   OP.subtract)
                    # T6 = 2 T3^2 - 1  (bf16 out; T3^2 as a vector multiply)
                    S3 = work.tile([128, HB], F32, name="S3", tag="S3")
                    nc.vector.tensor_mul(S3, T3, T3)
                    T6b = cheb.tile([128, HB], BF16, name="T6b", tag="T6b")
                    nc.vector.tensor_scalar(T6b, S3, 2.0, 1.0, OP.mult,
                                            OP.subtract)
                    # T7 = 2 T3 T4 - t  (bf16 out)
                    P = work.tile([128, HB], F32, name="P", tag="P")
                    nc.vector.tensor_mul(P, T3, T4)
                    T7b = cheb.tile([128, HB], BF16, name="T7b", tag="T7b")
                    nc.vector.scalar_tensor_tensor(T7b, P, 2.0, t, OP.mult,
                                                   OP.subtract)

                    Ts = (tb, T2b, T3b, T4b, T5b, T6b, T7b, T8b)
                    if half == 1 and it == IT - 1:
                        # Final it-block: jt-major order so each j-tile's
                        # accumulation finishes staggered and the PSUM
                        # drain copies/stores pipeline behind the
                        # remaining matmuls instead of all serializing
                        # after the last one.
                        for jt in range(JT):
                            for dm1, Td in enumerate(Ts):
                                k = it * 8 + dm1
                                nc.tensor.matmul(
                                    ps[jt],
                                    lhsT=ct[k][:, jt * 128:(jt + 1) * 128],
                                    rhs=Td,
                                    start=(k == 0),
                                    stop=(k == KT - 1),
                                )
                            ob = out_pool.tile([128, HB], F32, name="ob",
                                               tag="ob")
                            os_ = out[jt * 128:(jt + 1) * 128, hs]
                            if jt == JT - 1:
                                # Last drain is the serial tail: split the
                                # bias-add across Scalar+Vector and the DMA
                                # across two queues so it finishes in half
                                # the time.
                                nc.scalar.activation(
                                    ob[:, :HB // 2], ps[jt][:, :HB // 2],
                                    AF.Identity, bias=bias_all[:, jt:jt + 1])
                                nc.vector.tensor_scalar_add(
                                    ob[:, HB // 2:], ps[jt][:, HB // 2:],
                                    bias_all[:, jt:jt + 1])
                                nc.scalar.dma_start(
                                    out[jt * 128:(jt + 1) * 128,
                                        half * HB:half * HB + HB // 2],
                                    ob[:, :HB // 2])
                                nc.sync.dma_start(
                                    out[jt * 128:(jt + 1) * 128,
                                        half * HB + HB // 2:(half + 1) * HB],
                                    ob[:, HB // 2:])
                            elif jt % 2 == 0:
                                nc.scalar.activation(
                                    ob, ps[jt], AF.Identity,
                                    bias=bias_all[:, jt:jt + 1])
                                nc.scalar.dma_start(os_, ob)
                            else:
                                nc.vector.tensor_scalar_add(
                                    ob, ps[jt], bias_all[:, jt:jt + 1])
                                nc.sync.dma_start(os_, ob)
                    else:
                        for dm1, Td in enumerate(Ts):
                            k = it * 8 + dm1
                            for jt in range(JT):
                                nc.tensor.matmul(
                                    ps[jt],
                                    lhsT=ct[k][:, jt * 128:(jt + 1) * 128],
                                    rhs=Td,
                                    start=(k == 0),
                                    stop=(k == KT - 1),
                                )

                if half == 0:
                    # DMA issues ride the sync/gpsimd queues so the scalar
                    # queue (busy with the half-1 it-0 cheby chain) is not
                    # head-of-line blocked behind drain descriptors.
                    for jt in range(JT):
                        ob = out_pool.tile([128, HB], F32, name="ob",
                                           tag="ob")
                        if jt % 2 == 0:
                            nc.scalar.activation(ob, ps[jt], AF.Identity,
                                                 bias=bias_all[:, jt:jt + 1])
                            nc.sync.dma_start(
                                out[jt * 128:(jt + 1) * 128, hs], ob)
                        else:
                            nc.vector.tensor_scalar_add(
                                ob, ps[jt], bias_all[:, jt:jt + 1])
                            nc.gpsimd.dma_start(
                                out[jt * 128:(jt + 1) * 128, hs], ob)

    nc.compile()
    return nc


def _get_nc():
    if "nc" not in _CACHE:
        _CACHE["nc"] = _build_nc()
    return _CACHE["nc"]


def _prep_inputs(x, coefficients):
    x = np.asarray(x, dtype=np.float32)
    coefficients = np.asarray(coefficients, dtype=np.float32)
    xt_full = np.ascontiguousarray(x.T)  # [768, 8192]

    # coeffK[k = it*8+(d-1)] = coefficients[it*128:(it+1)*128, :, d]
    cr = coefficients.reshape(IT, 128, J, D1)
    arr = np.transpose(cr[:, :, :, 1:], (0, 3, 1, 2))  # [6, 8, 128, 768]
    coeff_in = np.ascontiguousarray(
        arr.reshape(KT, 128, J).astype(ml_dtypes.bfloat16))

    bias_in = np.ascontiguousarray(
        coefficients[:, :, 0].sum(axis=0).astype(np.float32).reshape(JT, 128).T
    )

    in_maps = []
    for c in range(NCORES):
        xt_c = np.ascontiguousarray(xt_full[:, c * BPC:(c + 1) * BPC])
        in_maps.append({"xt": xt_c, "coeff": coeff_in, "bias": bias_in})
    return in_maps


def _run(x, coefficients, trace=False, **run_kwargs):
    nc = _get_nc()
    in_maps = _prep_inputs(x, coefficients)
    res = bass_utils.run_bass_kernel_spmd(
        nc, in_maps, core_ids=list(range(NCORES)), trace=trace, **run_kwargs
    )
    out_full = np.empty((B, J), dtype=np.float32)
    for c in range(NCORES):
        out_full[c * BPC:(c + 1) * BPC, :] = res.results[c]["out"].T
    return out_full, res


def kernel(x, coefficients):
    out, _ = _run(x, coefficients, trace=False)
    return out


if __name__ == "__main__":
    rng = np.random.default_rng(0)
    x = rng.standard_normal((B, I), dtype=np.float32)
    std = 1.0 / (I * D1)
    coefficients = (std * rng.standard_normal((I, J, D1))).astype(np.float32)
    out = kernel(x, coefficients)
    print("out", out.shape, out.dtype, float(np.abs(out).mean()))


# revision 19
# speedup vs baseline: 1.0050x; 1.0050x over previous
"""ChebyKAN Trainium2 kernel.

Reference computation:
    t = tanh(x)                      # x: [8192, 768]
    cheby[b,i,d] = T_d(t[b,i])       # Chebyshev polys, d = 0..8
    out[b,j] = sum_{i,d} cheby[b,i,d] * coefficients[i,j,d]

Strategy (data-parallel over batch across 8 cores):
  - Each core gets a 1024-row batch shard, transposed on host to xt [768, 1024]
    so the contraction dim (in_features) lands on SBUF partitions.
  - out.T[j, b] = sum_k coeffK[k, j] * chebyK[k, b], K = 6*128 i-tiles x 8
    degrees (d=0 contributes a j-constant folded into a bias at PSUM drain).
  - bf16 matmuls: HW-measured 216 ns per K=128,N=512 matmul vs 227 ns for
    f32r (5% faster slot), quantization error ~2e-3 l2 vs the 2e-2 gate.
    fp8 DoubleRow (2x) was measured too but single-pass e4m3 error is
    3.3e-2 -> fails the gate; residual schemes cost 2x slots = no win.
  - Coefficients are cast to bf16 on host and kept SBUF-RESIDENT: one
    9.4 MB stream at kernel start (72 KB/partition) instead of streaming
    f32r twice (38 MB); DMA pressure is then trivial.
  - Two passes over batch halves of 512: per pass, all 6 j-tiles accumulate
    in 6 single-bank PSUM tiles over the 48 K-tiles; 576 matmuls total.
  - Chebyshev tiles via product identities: T2=2t^2-1, T3=2tT2-t, T4=2T2^2-1,
    T5=2T2T3-t, T6=2T3^2-1, T7=2T3T4-t, T8=2T4^2-1.  The even chain
    (t -> sq -> T2 -> sq2 -> T4 -> sq4 -> T8b) runs entirely on the Scalar
    engine with the x2-1 affines folded into activation scale/bias
    (func(in*2 - 1)), so the latency-critical recurrence has zero
    cross-engine hops; odd degrees + T6 run on Vector with bf16 outputs
    written directly by the final fused op.  The f32 recurrence keeps
    full precision; only the matmul operands are bf16.  No GpSimd
    compute (concurrent GpSimd elementwise steals DVE ports).
  - PE is warmed with a few dummy matmuls so HAM/p-state ramp happens
    before real work; real matmuls start as soon as tanh of the first
    tile lands (~9 us, incl. the ~7 us fixed NEFF prologue).
  - Scheduler sensitivity (HW-measured): bumping any tile-pool bufs
    beyond (work=3, cheb=2) reorders the Tile schedule and regresses
    3-6 us (drain/chain priority inversions); the ~430 ns/block slip of
    the cheby producers against the matmul stream is the stable
    equilibrium of the coupled in-order queues and costs ~2.4 us total.
    Device clock note: some runs execute at 2.0 GHz instead of 2.4
    (~172 us instead of ~142); this is environmental (power/thermal),
    not kernel-dependent -- rerun after idle to confirm regressions.
"""

import sys

for _p in ("/opt/trn_rl_repo",):
    if _p not in sys.path:
        sys.path.insert(0, _p)

import ml_dtypes
import numpy as np

import concourse.bass as bass
import concourse.mybir as mybir
import concourse.tile as tile
from concourse import bacc
from concourse import bass_utils
from concourse.tile import TileContext

F32 = mybir.dt.float32
BF16 = mybir.dt.bfloat16
AF = mybir.ActivationFunctionType
OP = mybir.AluOpType

B, I, J, D1 = 8192, 768, 768, 9  # batch, in_features, out_features, degree+1
NCORES = 8
BPC = B // NCORES      # 1024 batch rows per core
IT = I // 128          # 6 i-tiles
KT = IT * 8            # 48 K-tiles (d = 1..8)
JT = J // 128          # 6 j-tiles
HB = 512               # half-batch (matmul N)

_CACHE = {}


def _build_nc():
    nc = bacc.Bacc("TRN2", target_bir_lowering=False, debug=False,
                   num_devices=NCORES)
    xt = nc.dram_tensor("xt", [I, BPC], F32, kind="ExternalInput").ap()
    # coeff[k, i, j]: K-tile k = it*8 + (d-1)
    coeff = nc.dram_tensor("coeff", [KT, 128, J], BF16,
                           kind="ExternalInput").ap()
    bias = nc.dram_tensor("bias", [128, JT], F32, kind="ExternalInput").ap()
    out = nc.dram_tensor("out", [J, BPC], F32, kind="ExternalOutput").ap()

    with TileContext(nc) as tc:
        with (
            tc.tile_pool(name="xtp", bufs=1) as xt_pool,
            tc.tile_pool(name="work", bufs=3) as work,
            tc.tile_pool(name="chebp", bufs=2) as cheb,
            tc.tile_pool(name="coeffp", bufs=1) as coeff_pool,
            tc.tile_pool(name="outp", bufs=6) as out_pool,
            tc.tile_pool(name="biasp", bufs=1) as bias_pool,
            tc.tile_pool(name="psum", bufs=8, space="PSUM") as psum_pool,
        ):
            # PE warm-up scratch (HAM/p-state ramp before real matmuls).
            warm = work.tile([128, HB], BF16, name="warm", tag="warm", bufs=1)
            nc.vector.memset(warm, 0.0)

            bias_all = bias_pool.tile([128, JT], F32, name="bias_all",
                                      tag="bias_all")
            neg1 = bias_pool.tile([128, 1], F32, name="neg1", tag="neg1")
            nc.vector.memset(neg1, -1.0)

            # Resident coefficient tiles + input tiles; DMA order = first
            # consumed first.  The first xt tile is split so the half-0
            # columns (what the first tanh reads) land in ~0.7 us instead
            # of 1.4, and coeff k=0 rides right behind them.
            xt_tiles = [None] * IT
            ct = [None] * KT

            def coeff_tile(k):
                c = coeff_pool.tile([128, J], BF16, name=f"c{k}", tag=f"c{k}")
                nc.sync.dma_start(c, coeff[k])
                ct[k] = c

            xt0 = xt_pool.tile([128, BPC], F32, name="xtt0", tag="xtt0")
            nc.sync.dma_start(xt0[:, :HB], xt[:128, :HB])
            xt_tiles[0] = xt0
            coeff_tile(0)
            nc.sync.dma_start(xt0[:, HB:], xt[:128, HB:])
            for dm1 in range(1, 8):
                coeff_tile(dm1)
            for it in range(1, IT):
                xtt = xt_pool.tile([128, BPC], F32, name=f"xtt{it}",
                                   tag=f"xtt{it}")
                nc.sync.dma_start(xtt, xt[it * 128:(it + 1) * 128, :])
                xt_tiles[it] = xtt
                for dm1 in range(8):
                    coeff_tile(it * 8 + dm1)
                if it == 1:
                    nc.sync.dma_start(bias_all, bias)

            for half in range(2):
                hs = slice(half * HB, (half + 1) * HB)
                ps = [psum_pool.tile([128, HB], F32, name="ps", tag="ps")
                      for _ in range(JT)]
                if half == 0:
                    # dummy matmuls into ps[0]; overwritten by the real
                    # k==0 matmul (start=True clears has_written)
                    for _ in range(7):
                        nc.tensor.matmul(ps[0], lhsT=warm[:, :128], rhs=warm,
                                         start=True, stop=True)

                for it in range(IT):
                    xs = xt_tiles[it][:, hs]
                    # In steady state the f32 chain (t -> squares) is the
                    # critical path, so emit tanh->t first; tb (consumed at
                    # the very start of the block) has a whole block of
                    # production slack -- except for the first block, where
                    # tb gates the first real matmul.
                    tb = cheb.tile([128, HB], BF16, name="tb", tag="tb")
                    t = work.tile([128, HB], F32, name="t", tag="t")
                    if half == 0 and it == 0:
                        nc.scalar.activation(tb, xs, AF.Tanh)
                        nc.scalar.activation(t, xs, AF.Tanh)
                    else:
                        nc.scalar.activation(t, xs, AF.Tanh)
                        nc.scalar.activation(tb, xs, AF.Tanh)
                    # Even chain runs entirely on the Scalar engine with the
                    # x2-1 affines folded into activation scale/bias --
                    # t -> sq -> T2 -> sq2 -> T4 -> sq4 back-to-back on one
                    # in-order queue, no cross-engine hops.
                    sq = work.tile([128, HB], F32, name="sq", tag="sq")
                    nc.scalar.activation(sq, t, AF.Square)
                    T2 = work.tile([128, HB], F32, name="T2", tag="T2")
                    nc.scalar.activation(T2, sq, AF.Identity, bias=neg1,
                                         scale=2.0)
                    T2b = cheb.tile([128, HB], BF16, name="T2b", tag="T2b")
                    nc.scalar.activation(T2b, sq, AF.Identity, bias=neg1,
                                         scale=2.0)
                    sq2 = work.tile([128, HB], F32, name="sq2", tag="sq2")
                    nc.scalar.activation(sq2, T2, AF.Square)
                    T4 = work.tile([128, HB], F32, name="T4", tag="T4")
                    nc.scalar.activation(T4, sq2, AF.Identity, bias=neg1,
                                         scale=2.0)
                    T4b = cheb.tile([128, HB], BF16, name="T4b", tag="T4b")
                    nc.scalar.activation(T4b, sq2, AF.Identity, bias=neg1,
                                         scale=2.0)
                    sq4 = work.tile([128, HB], F32, name="sq4", tag="sq4")
                    nc.scalar.activation(sq4, T4, AF.Square)
                    T8b = cheb.tile([128, HB], BF16, name="T8b", tag="T8b")
                    nc.scalar.activation(T8b, sq4, AF.Identity, bias=neg1,
                                         scale=2.0)
                    # Odd degrees + T6 on the Vector engine.
                    # T3 = 2 t T2 - t
                    P = work.tile([128, HB], F32, name="P", tag="P")
                    nc.vector.tensor_mul(P, t, T2)
                    T3 = work.tile([128, HB], F32, name="T3", tag="T3")
                    nc.vector.scalar_tensor_tensor(T3, P, 2.0, t, OP.mult,
                                                   OP.subtract)
                    T3b = cheb.tile([128, HB], BF16, name="T3b", tag="T3b")
                    nc.vector.tensor_copy(T3b, T3)
                    # T5 = 2 T2 T3 - t  (bf16 out)
                    P = work.tile([128, HB], F32, name="P", tag="P")
                    nc.vector.tensor_mul(P, T2, T3)
                    T5b = cheb.tile([128, HB], BF16, name="T5b", tag="T5b")
                    nc.vector.scalar_tensor_tensor(T5b, P, 2.0, t, OP.mult,
                                                   OP.subtract)
                    # T6 = 2 T3^2 - 1  (bf16 out; T3^2 as a vector multiply)
                    S3 = work.tile([128, HB], F32, name="S3", tag="S3")
                    nc.vector.tensor_mul(S3, T3, T3)
                    T6b = cheb.tile([128, HB], BF16, name="T6b", tag="T6b")
                    nc.vector.tensor_scalar(T6b, S3, 2.0, 1.0, OP.mult,
                                            OP.subtract)
                    # T7 = 2 T3 T4 - t  (bf16 out)
                    P = work.tile([128, HB], F32, name="P", tag="P")
                    nc.vector.tensor_mul(P, T3, T4)
                    T7b = cheb.tile([128, HB], BF16, name="T7b", tag="T7b")
                    nc.vector.scalar_tensor_tensor(T7b, P, 2.0, t, OP.mult,
                                                   OP.subtract)

                    Ts = (tb, T2b, T3b, T4b, T5b, T6b, T7b, T8b)
                    if half == 1 and it == IT - 1:
                        # Final it-block: jt-major order so each j-tile's
                        # accumulation finishes staggered and the PSUM
                        # drain copies/stores pipeline behind the
                        # remaining matmuls instead of all serializing
                        # after the last one.
                        for jt in range(JT):
                            for dm1, Td in enumerate(Ts):
                                k = it * 8 + dm1
                                nc.tensor.matmul(
                                    ps[jt],
                                    lhsT=ct[k][:, jt * 128:(jt + 1) * 128],
                                    rhs=Td,
                                    start=(k == 0),
                                    stop=(k == KT - 1),
                                )
                            ob = out_pool.tile([128, HB], F32, name="ob",
                                               tag="ob")
                            if jt % 2 == 0:
                                nc.scalar.activation(
                                    ob, ps[jt], AF.Identity,
                                    bias=bias_all[:, jt:jt + 1])
                                nc.scalar.dma_start(
                                    out[jt * 128:(jt + 1) * 128, hs], ob)
                            else:
                                nc.vector.tensor_scalar_add(
                                    ob, ps[jt], bias_all[:, jt:jt + 1])
                                nc.sync.dma_start(
                                    out[jt * 128:(jt + 1) * 128, hs], ob)
                    else:
                        for dm1, Td in enumerate(Ts):
                            k = it * 8 + dm1
                            for jt in range(JT):
                                nc.tensor.matmul(
                                    ps[jt],
                                    lhsT=ct[k][:, jt * 128:(jt + 1) * 128],
                                    rhs=Td,
                                    start=(k == 0),
                                    stop=(k == KT - 1),
                                )

                if half == 0:
                    # DMA issues ride the sync/gpsimd queues so the scalar
                    # queue (busy with the half-1 it-0 cheby chain) is not
                    # head-of-line blocked behind drain descriptors.
                    for jt in range(JT):
                        ob = out_pool.tile([128, HB], F32, name="ob",
                                           tag="ob")
                        if jt % 2 == 0:
                            nc.scalar.activation(ob, ps[jt], AF.Identity,
                                                 bias=bias_all[:, jt:jt + 1])
                            nc.sync.dma_start(
                                out[jt * 128:(jt + 1) * 128, hs], ob)
                        else:
                            nc.vector.tensor_scalar_add(
                                ob, ps[jt], bias_all[:, jt:jt + 1])
                            nc.gpsimd.dma_start(
                                out[jt * 128:(jt + 1) * 128, hs], ob)

    nc.compile()
    return nc


def _get_nc():
    if "nc" not in _CACHE:
        _CACHE["nc"] = _build_nc()
    return _CACHE["nc"]


def _prep_inputs(x, coefficients):
    x = np.asarray(x, dtype=np.float32)
    coefficients = np.asarray(coefficients, dtype=np.float32)
    xt_full = np.ascontiguousarray(x.T)  # [768, 8192]

    # coeffK[k = it*8+(d-1)] = coefficients[it*128:(it+1)*128, :, d]
    cr = coefficients.reshape(IT, 128, J, D1)
    arr = np.transpose(cr[:, :, :, 1:], (0, 3, 1, 2))  # [6, 8, 128, 768]
    coeff_in = np.ascontiguousarray(
        arr.reshape(KT, 128, J).astype(ml_dtypes.bfloat16))

    bias_in = np.ascontiguousarray(
        coefficients[:, :, 0].sum(axis=0).astype(np.float32).reshape(JT, 128).T
    )

    in_maps = []
    for c in range(NCORES):
        xt_c = np.ascontiguousarray(xt_full[:, c * BPC:(c + 1) * BPC])
        in_maps.append({"xt": xt_c, "coeff": coeff_in, "bias": bias_in})
    return in_maps


def _run(x, coefficients, trace=False, **run_kwargs):
    nc = _get_nc()
    in_maps = _prep_inputs(x, coefficients)
    res = bass_utils.run_bass_kernel_spmd(
        nc, in_maps, core_ids=list(range(NCORES)), trace=trace, **run_kwargs
    )
    out_full = np.empty((B, J), dtype=np.float32)
    for c in range(NCORES):
        out_full[c * BPC:(c + 1) * BPC, :] = res.results[c]["out"].T
    return out_full, res


def kernel(x, coefficients):
    out, _ = _run(x, coefficients, trace=False)
    return out


if __name__ == "__main__":
    rng = np.random.default_rng(0)
    x = rng.standard_normal((B, I), dtype=np.float32)
    std = 1.0 / (I * D1)
    coefficients = (std * rng.standard_normal((I, J, D1))).astype(np.float32)
    out = kernel(x, coefficients)
    print("out", out.shape, out.dtype, float(np.abs(out).mean()))
